# revision 40
# baseline (speedup 1.0000x reference)
"""MoE (top-2 of 8 experts, D=1024, F=2048, T=4096) on 8 Trainium2 NeuronCores.

Strategy: expert-parallel. Every core replicates the fp32 router over all
4096 tokens, selects the tokens routed to ITS expert (top-2 membership via
max8 on logits; weights w1=sigmoid(l1-l2) renormalized pair weights),
compacts their indices with a matmul-based exclusive cumsum + indirect-DMA
scatter, gathers those token rows, runs the gated-MLP for its single expert
in bf16 (fp32 accumulate), scales each token's output row by its routing
weight, and writes a compact [CAP, D] fp32 result + the slot->token map.
The host sums the 8 compact shards into the full [T, D] output. Router
logits are computed in fp32 on-device and returned from core 0.
"""

import os
import sys

import numpy as np
import ml_dtypes

if "/opt/trn_rl_repo" not in sys.path:
    sys.path.insert(0, "/opt/trn_rl_repo")

# Problem shapes (hardcoded per contract)
T, D, F, E = 4096, 1024, 2048, 8
P = 128
NT = T // P            # 32 token tiles of 128
REG = 320              # slots per token-quarter region (mean 256, ~4.5 sigma margin)
CAP = 4 * REG          # per-expert token capacity
NJ = CAP // P          # capacity tiles
GROUPS = [(0, 512), (512, 512), (1024, 256)]  # (slot offset, group size)
KO = D // P            # 8 contraction chunks over D
KI = F // P            # 16 contraction chunks over F
DP = D + 32            # compact row width, 64B-aligned (w_hi/w_lo at D, D+1)
BIG = 100000           # position sentinel for unselected tokens
                       # (BIG * row-stride must stay well inside int32)

N_CORES = 8

_CACHE = {}


def _build_nc():
    import concourse.tile as tile
    from concourse import bacc, mybir
    from concourse.bass import IndirectOffsetOnAxis

    f32 = mybir.dt.float32
    bf = mybir.dt.bfloat16
    i32 = mybir.dt.int32
    AF = mybir.ActivationFunctionType
    AX = mybir.AxisListType
    OP = mybir.AluOpType

    nc = bacc.Bacc("TRN2", target_bir_lowering=False, debug=False,
                   enable_asserts=False, num_devices=N_CORES)

    # ---- I/O ----
    xt_d = nc.dram_tensor("x_t", [D, T], f32, kind="ExternalInput").ap()
    xb_d = nc.dram_tensor("x_bf", [T, D], bf, kind="ExternalInput").ap()
    wr_d = nc.dram_tensor("w_r", [D, E], f32, kind="ExternalInput").ap()
    wg_d = nc.dram_tensor("w_g", [D, F], bf, kind="ExternalInput").ap()
    wi_d = nc.dram_tensor("w_i", [D, F], bf, kind="ExternalInput").ap()
    wo_d = nc.dram_tensor("w_o", [F, D], bf, kind="ExternalInput").ap()
    sel_d = nc.dram_tensor("sel", [P, E], f32, kind="ExternalInput").ap()
    u128_d = nc.dram_tensor("u128", [P, P], f32, kind="ExternalInput").ap()
    u32_d = nc.dram_tensor("u32", [NT, NT], f32, kind="ExternalInput").ap()
    idf_d = nc.dram_tensor("idf", [P, P], f32, kind="ExternalInput").ap()
    idb_d = nc.dram_tensor("idb", [P, P], bf, kind="ExternalInput").ap()
    iota_d = nc.dram_tensor("iota", [P, NT], f32, kind="ExternalInput").ap()

    lg_out = nc.dram_tensor("logits_out", [T, E], f32, kind="ExternalOutput").ap()
    y_out = nc.dram_tensor("y_out", [CAP, D], f32, kind="ExternalOutput").ap()
    metas = [(nc.dram_tensor(f"meta_q{r}a", [REG, 2], f32).ap(),
              nc.dram_tensor(f"meta_q{r}b", [REG, 2], f32).ap())
             for r in range(4)]

    with tile.TileContext(nc) as tc:
        from contextlib import ExitStack
        with ExitStack() as ctx:
            consts = ctx.enter_context(tc.tile_pool(name="consts", bufs=1))
            wpool = ctx.enter_context(tc.tile_pool(name="wpool", bufs=1))
            xtp = ctx.enter_context(tc.tile_pool(name="xtp", bufs=5))
            rsm = ctx.enter_context(tc.tile_pool(name="rsm", bufs=2))
            asmp = ctx.enter_context(tc.tile_pool(name="asm", bufs=1))
            gthp = ctx.enter_context(tc.tile_pool(name="gth", bufs=4))
            xgp = ctx.enter_context(tc.tile_pool(name="xgp", bufs=2))
            mlpp = ctx.enter_context(tc.tile_pool(name="mlp", bufs=2))
            ytp = ctx.enter_context(tc.tile_pool(name="ytp", bufs=2))
            psum_mm = ctx.enter_context(tc.tile_pool(name="psmm", bufs=5, space="PSUM"))
            psum_sm = ctx.enter_context(tc.tile_pool(name="pssm", bufs=2, space="PSUM"))
            psum_tr = ctx.enter_context(tc.tile_pool(name="pstr", bufs=1, space="PSUM"))

            # ---- router-critical constant ----
            wr_sb = consts.tile([P, KO, E], f32)
            nc.sync.dma_start(wr_sb, wr_d.rearrange("(ko p) e -> p ko e", p=P))
            idf_sb = consts.tile([P, P], f32)
            nc.scalar.dma_start(idf_sb, idf_d)

            # warm the PE clock (HAM) during the first x_t load's latency:
            # ~5us of tiny matmuls on the already-loaded router weights.
            wu_ps = psum_mm.tile([E, 512], f32, tag="mm", name="warm")
            for i in range(56):
                nc.tensor.matmul(wu_ps[:, :E], lhsT=wr_sb[:, i % KO],
                                 rhs=wr_sb[:, (i + 1) % KO],
                                 start=(i == 0), stop=(i == 55))
            wu_sb = rsm.tile([E, E], f32, tag="wusb")
            nc.vector.tensor_copy(wu_sb, wu_ps[:, :E])

            # ---- router (fp32) + per-half routing/compaction ----
            lgall = asmp.tile([P, NT, E], f32)    # logits, token t = c*128 + p
            l12 = asmp.tile([P, NT, 8], f32)      # max8 sorted logits per tile
            w_all = asmp.tile([P, NT], f32)
            m_all = asmp.tile([P, NT], f32)
            pos_i = asmp.tile([P, NT], i32)
            sc = asmp.tile([P, NT, 2], f32)
            mz = asmp.tile([64, 2 * REG // 64], f32)
            xt_re = xt_d.rearrange("(ko p) t -> p ko t", p=P)
            TQ = 1024   # tokens per router sweep (row-contiguous DMA)

            def router_quarter(q):
                pss = [psum_mm.tile([E, 512], f32, tag="mm", name=f"pslt{q}_{th}")
                       for th in range(TQ // 512)]
                for ko in range(KO):
                    xt_g = xtp.tile([P, TQ], f32, tag="xtg", name=f"xtg{q}_{ko}")
                    nc.sync.dma_start(xt_g, xt_re[:, ko, q * TQ:(q + 1) * TQ])
                    for th in range(TQ // 512):
                        nc.tensor.matmul(pss[th], lhsT=wr_sb[:, ko],
                                         rhs=xt_g[:, th * 512:(th + 1) * 512],
                                         start=(ko == 0), stop=(ko == KO - 1))
                for th in range(TQ // 512):
                    lgt = rsm.tile([E, 512], f32, tag="lgt")
                    nc.vector.tensor_copy(lgt, pss[th])
                    for jj in range(4):
                        j = (q * TQ + th * 512) // P + jj
                        ps_l = psum_sm.tile([P, E], f32, tag="small")
                        nc.tensor.transpose(ps_l, lgt[:, jj * P:(jj + 1) * P],
                                            idf_sb[:E, :E])
                        nc.vector.tensor_copy(lgall[:, j], ps_l)
                        nc.vector.max(l12[:, j], lgall[:, j])

            NH = NT // 4

            def route_half(h):
                cs = slice(h * NH, (h + 1) * NH)
                l1 = l12[:, cs, 0]
                l2 = l12[:, cs, 1]
                d21 = rsm.tile([P, NH], f32, tag="d21")
                nc.vector.tensor_sub(d21, l2, l1)
                w2a = rsm.tile([P, NH], f32, tag="w2a")
                nc.scalar.activation(w2a, d21, AF.Sigmoid)   # w2 = sig(l2 - l1)
                w1a = rsm.tile([P, NH], f32, tag="w1a")
                nc.vector.tensor_scalar(w1a, w2a, -1.0, 1.0, op0=OP.mult, op1=OP.add)
                msel = rsm.tile([P, NH, E], f32, tag="msel")
                nc.vector.tensor_mul(msel, lgall[:, cs],
                                     sel_sb[:, None, :].to_broadcast([P, NH, E]))
                le = rsm.tile([P, NH], f32, tag="le")
                nc.vector.reduce_sum(le, msel, axis=AX.X)
                m1 = rsm.tile([P, NH], f32, tag="m1")
                nc.vector.tensor_tensor(m1, le, l1, op=OP.is_equal)
                m2 = rsm.tile([P, NH], f32, tag="m2")
                nc.vector.tensor_tensor(m2, le, l2, op=OP.is_equal)
                t1 = rsm.tile([P, NH], f32, tag="t1")
                nc.vector.tensor_mul(t1, m1, w1a)
                t2 = rsm.tile([P, NH], f32, tag="t2")
                nc.vector.tensor_mul(t2, m2, w2a)
                nc.vector.tensor_add(w_all[:, cs], t1, t2)
                nc.vector.tensor_add(m_all[:, cs], m1, m2)
                nc.vector.tensor_scalar_min(m_all[:, cs], m_all[:, cs], 1.0)
                nc.vector.tensor_copy(sc[:, cs, 1], w_all[:, cs])
                # within-column exclusive cumsum (transposed) + column offsets
                ps_se = psum_sm.tile([NH, P], f32, tag="small")
                nc.tensor.matmul(ps_se, lhsT=m_all[:, cs], rhs=u128_sb,
                                 start=True, stop=True)
                se_sb = rsm.tile([NH, P], f32, tag="sesb")
                nc.vector.tensor_copy(se_sb, ps_se)
                ps_mt = psum_sm.tile([NH, P], f32, tag="small")
                nc.tensor.transpose(ps_mt, m_all[:, cs], idf_sb)
                mt = rsm.tile([NH, P], f32, tag="mt")
                nc.vector.tensor_copy(mt, ps_mt)
                csum_h = rsm.tile([NH, 1], f32, tag="csum")
                nc.vector.reduce_sum(csum_h, mt, axis=AX.X)
                ps_off = psum_sm.tile([NH, 1], f32, tag="small")
                nc.tensor.matmul(ps_off, lhsT=u32_sb[:NH, :NH], rhs=csum_h,
                                 start=True, stop=True)
                offs_h = rsm.tile([NH, 1], f32, tag="offs")
                nc.vector.tensor_copy(offs_h, ps_off)
                posT = rsm.tile([NH, P], f32, tag="posT")
                nc.vector.tensor_scalar_add(posT, se_sb, offs_h)
                ps_pos = psum_sm.tile([P, NH], f32, tag="small")
                nc.tensor.transpose(ps_pos, posT, idf_sb[:NH, :NH])
                pm = rsm.tile([P, NH], f32, tag="pm")
                nc.vector.tensor_mul(pm, ps_pos, m_all[:, cs])
                bigt = rsm.tile([P, NH], f32, tag="bigt")
                nc.vector.tensor_scalar(bigt, m_all[:, cs], -float(BIG), float(BIG),
                                        op0=OP.mult, op1=OP.add)
                nc.vector.tensor_add(pm, pm, bigt)
                nc.vector.tensor_copy(pos_i[:, cs], pm)
                # compact-slot scatter of (token_id, weight)
                with nc.allow_non_contiguous_dma(reason="8B-row compaction scatter"):
                    for c in range(h * NH, (h + 1) * NH):
                        nc.gpsimd.indirect_dma_start(
                            out=metas[h][c % 2],
                            out_offset=IndirectOffsetOnAxis(
                                ap=pos_i[:, c:c + 1], axis=0),
                            in_=sc[:, c, :],
                            in_offset=None,
                            bounds_check=REG - 1,
                            oob_is_err=False,
                        )

            router_quarter(0)

            # remaining constants (off the router-warmup critical path)
            sel_sb = consts.tile([P, E], f32)
            nc.sync.dma_start(sel_sb, sel_d)
            u128_sb = consts.tile([P, P], f32)
            nc.sync.dma_start(u128_sb, u128_d)
            u32_sb = consts.tile([NT, NT], f32)
            nc.sync.dma_start(u32_sb, u32_d)
            idb_sb = consts.tile([P, P], bf)
            nc.sync.dma_start(idb_sb, idb_d)
            iota_sb = consts.tile([P, NT], f32)
            nc.vector.memset(mz, 0.0)
            nc.sync.dma_start(iota_sb, iota_d)
            for r in range(4):
                for mq in metas[r]:
                    nc.scalar.dma_start(
                        mq.rearrange("(o p) b -> p o b", p=64),
                        mz[:64].rearrange("p (o b) -> p o b", b=2))
            nc.vector.tensor_copy(sc[:, :, 0], iota_sb)

            router_quarter(1)
            route_half(0)       # each block overlaps the next router sweep
            router_quarter(2)
            route_half(1)
            router_quarter(3)
            route_half(2)

            # ---- expert weight preload (bf16, stays resident) ----
            wg_sb = wpool.tile([P, KO, F], bf)
            nc.scalar.dma_start(wg_sb, wg_d.rearrange("(ko p) f -> p ko f", p=P))
            wi_sb = wpool.tile([P, KO, F], bf)
            nc.scalar.dma_start(wi_sb, wi_d.rearrange("(ko p) f -> p ko f", p=P))
            wo_sb = wpool.tile([P, KI, D], bf)
            nc.scalar.dma_start(wo_sb, wo_d.rearrange("(ki p) d -> p ki d", p=P))

            # logits output (token-major [T, E])
            nc.scalar.dma_start(lg_out.rearrange("(c p) e -> p c e", p=P), lgall)

            route_half(3)

            # ---- expert MLP over capacity tiles ----
            wv = asmp.tile([P, NJ], f32)   # per-slot routing weight
            for g, (t0, ng) in enumerate(GROUPS):
                njg = ng // P
                xtg = xgp.tile([P, KO, 512], bf, tag="xtgrp")
                for jj in range(njg):
                    j = t0 // P + jj
                    meta_t = gthp.tile([P, 2], f32, tag="meta")
                    meta_t2 = gthp.tile([P, 2], f32, tag="meta2")
                    s0, s1 = j * P, (j + 1) * P
                    with nc.allow_non_contiguous_dma(reason="8B meta rows"):
                        for r in range(s0 // REG, (s1 - 1) // REG + 1):
                            l0 = max(s0, r * REG) - r * REG
                            l1 = min(s1, (r + 1) * REG) - r * REG
                            o0 = r * REG + l0 - s0
                            nc.sync.dma_start(meta_t[o0:o0 + l1 - l0, :],
                                              metas[r][0][l0:l1, :])
                            nc.sync.dma_start(meta_t2[o0:o0 + l1 - l0, :],
                                              metas[r][1][l0:l1, :])
                    nc.vector.tensor_add(meta_t, meta_t, meta_t2)
                    idx_i = gthp.tile([P, 1], i32, tag="idx")
                    nc.vector.tensor_copy(idx_i, meta_t[:, 0:1])
                    nc.vector.tensor_copy(wv[:, j:j + 1], meta_t[:, 1:2])
                    xg = gthp.tile([P, D], bf, tag="xg")
                    nc.gpsimd.indirect_dma_start(
                        out=xg, out_offset=None, in_=xb_d,
                        in_offset=IndirectOffsetOnAxis(ap=idx_i[:, 0:1], axis=0))
                    for ko in range(KO):
                        ps_tr = psum_tr.tile([P, P], bf, tag="trb")
                        nc.tensor.transpose(ps_tr, xg[:, ko * P:(ko + 1) * P], idb_sb)
                        nc.vector.tensor_copy(xtg[:, ko, jj * P:(jj + 1) * P], ps_tr)

                # gate/up + silu + mul (act kept in gsil, bf16)
                gsil = mlpp.tile([P, KI, 512], bf, tag="gsil")
                for m in range(KI):
                    ps = psum_mm.tile([P, 512], f32, tag="mm")
                    for ko in range(KO):
                        nc.tensor.matmul(ps[:, :ng], lhsT=wg_sb[:, ko, m * P:(m + 1) * P],
                                         rhs=xtg[:, ko, :ng], start=(ko == 0),
                                         stop=(ko == KO - 1))
                    nc.scalar.activation(gsil[:, m, :ng], ps[:, :ng], AF.Sigmoid)
                    nc.vector.tensor_mul(gsil[:, m, :ng], ps[:, :ng], gsil[:, m, :ng])
                    ps2 = psum_mm.tile([P, 512], f32, tag="mm")
                    for ko in range(KO):
                        nc.tensor.matmul(ps2[:, :ng], lhsT=wi_sb[:, ko, m * P:(m + 1) * P],
                                         rhs=xtg[:, ko, :ng], start=(ko == 0),
                                         stop=(ko == KO - 1))
                    nc.vector.tensor_mul(gsil[:, m, :ng], ps2[:, :ng], gsil[:, m, :ng])

                # down proj + transpose back + per-token scale + store
                for do in range(KO):
                    ps3 = psum_mm.tile([P, 512], f32, tag="mm")
                    for ki in range(KI):
                        nc.tensor.matmul(ps3[:, :ng], lhsT=wo_sb[:, ki, do * P:(do + 1) * P],
                                         rhs=gsil[:, ki, :ng], start=(ki == 0),
                                         stop=(ki == KI - 1))
                    ysb = ytp.tile([P, 512], f32, tag="ysb")
                    nc.vector.tensor_copy(ysb[:, :ng], ps3[:, :ng])
                    for jj in range(njg):
                        j = t0 // P + jj
                        ps4 = psum_sm.tile([P, P], f32, tag="small")
                        nc.tensor.transpose(ps4, ysb[:, jj * P:(jj + 1) * P], idf_sb)
                        ystg = ytp.tile([P, P], f32, tag="ystg")
                        nc.vector.tensor_scalar_mul(ystg, ps4, wv[:, j:j + 1])
                        nc.sync.dma_start(
                            y_out[j * P:(j + 1) * P, do * P:(do + 1) * P], ystg)

    nc.compile()
    return nc


def _get_nc():
    if "nc" not in _CACHE:
        _CACHE["nc"] = _build_nc()
    return _CACHE["nc"]


def _make_in_maps(x, W_router, W_gate, W_in, W_out):
    bf16 = ml_dtypes.bfloat16
    x2d = np.ascontiguousarray(x.reshape(T, D).astype(np.float32))
    x_t = np.ascontiguousarray(x2d.T)
    x_bf = np.ascontiguousarray(x2d.astype(bf16))
    wr = np.ascontiguousarray(W_router.astype(np.float32))
    u128 = np.triu(np.ones((P, P), np.float32), 1)
    u32 = np.triu(np.ones((NT, NT), np.float32), 1)
    idf = np.eye(P, dtype=np.float32)
    idb = np.eye(P, dtype=np.float32).astype(bf16)
    iota = np.ascontiguousarray(
        (np.arange(P, dtype=np.float32)[:, None]
         + P * np.arange(NT, dtype=np.float32)[None, :]).astype(np.float32))

    in_maps = []
    for e in range(N_CORES):
        sel = np.zeros((P, E), np.float32)
        sel[:, e] = 1.0
        in_maps.append({
            "x_t": x_t,
            "x_bf": x_bf,
            "w_r": wr,
            "w_g": np.ascontiguousarray(W_gate[e].astype(bf16)),
            "w_i": np.ascontiguousarray(W_in[e].astype(bf16)),
            "w_o": np.ascontiguousarray(W_out[e].astype(bf16)),
            "sel": sel,
            "u128": u128,
            "u32": u32,
            "idf": idf,
            "idb": idb,
            "iota": iota,
        })
    return in_maps


def kernel(x, W_router, W_gate, W_in, W_out, _trace=False, _trace_cores=None):
    from concourse.bass_utils import run_bass_kernel_spmd

    nc = _get_nc()
    in_maps = _make_in_maps(x, W_router, W_gate, W_in, W_out)
    res = run_bass_kernel_spmd(nc, in_maps, list(range(N_CORES)),
                               trace=_trace, trace_cores=_trace_cores)
    kernel._last_results = res

    logits = np.asarray(res.results[0]["logits_out"], dtype=np.float32)
    # Replicate the device's top-2 membership mask bit-exactly from the same
    # fp32 logits the device routed with; slot order == ascending token id.
    srt = np.sort(logits, axis=1)[:, ::-1]
    l1, l2 = srt[:, 0:1], srt[:, 1:2]
    member = (logits == l1) | (logits == l2)      # [T, E]
    out = np.zeros((T, D), np.float32)
    for e in range(N_CORES):
        y = np.asarray(res.results[e]["y_out"])
        for r in range(4):
            mem_r = member[r * (T // 4):(r + 1) * (T // 4), e]
            idx = np.nonzero(mem_r)[0][:REG] + r * (T // 4)
            out[idx] += y[r * REG:r * REG + len(idx)]
    B, S = 2, 2048
    return out.reshape(B, S, D), logits


# revision 41
# speedup vs baseline: 1.0115x; 1.0115x over previous
"""MoE (top-2 of 8 experts, D=1024, F=2048, T=4096) on 8 Trainium2 NeuronCores.

Strategy: expert-parallel. Every core replicates the fp32 router over all
4096 tokens, selects the tokens routed to ITS expert (top-2 membership via
max8 on logits; weights w1=sigmoid(l1-l2) renormalized pair weights),
compacts their indices with a matmul-based exclusive cumsum + indirect-DMA
scatter, gathers those token rows, runs the gated-MLP for its single expert
in bf16 (fp32 accumulate), scales each token's output row by its routing
weight, and writes a compact [CAP, D] fp32 result + the slot->token map.
The host sums the 8 compact shards into the full [T, D] output. Router
logits are computed in fp32 on-device and returned from core 0.
"""

import os
import sys

import numpy as np
import ml_dtypes

if "/opt/trn_rl_repo" not in sys.path:
    sys.path.insert(0, "/opt/trn_rl_repo")

# Problem shapes (hardcoded per contract)
T, D, F, E = 4096, 1024, 2048, 8
P = 128
NT = T // P            # 32 token tiles of 128
REG = 320              # slots per token-quarter region (mean 256, ~4.5 sigma margin)
CAP = 4 * REG          # per-expert token capacity
NJ = CAP // P          # capacity tiles
GROUPS = [(0, 512), (512, 512), (1024, 256)]  # (slot offset, group size)
KO = D // P            # 8 contraction chunks over D
KI = F // P            # 16 contraction chunks over F
DP = D + 32            # compact row width, 64B-aligned (w_hi/w_lo at D, D+1)
BIG = 100000           # position sentinel for unselected tokens
                       # (BIG * row-stride must stay well inside int32)

N_CORES = 8

_CACHE = {}


def _build_nc():
    import concourse.tile as tile
    from concourse import bacc, mybir
    from concourse.bass import IndirectOffsetOnAxis

    f32 = mybir.dt.float32
    bf = mybir.dt.bfloat16
    i32 = mybir.dt.int32
    AF = mybir.ActivationFunctionType
    AX = mybir.AxisListType
    OP = mybir.AluOpType

    nc = bacc.Bacc("TRN2", target_bir_lowering=False, debug=False,
                   enable_asserts=False, num_devices=N_CORES)

    # ---- I/O ----
    xt_d = nc.dram_tensor("x_t", [D, T], f32, kind="ExternalInput").ap()
    xb_d = nc.dram_tensor("x_bf", [T, D], bf, kind="ExternalInput").ap()
    wr_d = nc.dram_tensor("w_r", [D, E], f32, kind="ExternalInput").ap()
    wg_d = nc.dram_tensor("w_g", [D, F], bf, kind="ExternalInput").ap()
    wi_d = nc.dram_tensor("w_i", [D, F], bf, kind="ExternalInput").ap()
    wo_d = nc.dram_tensor("w_o", [F, D], bf, kind="ExternalInput").ap()
    sel_d = nc.dram_tensor("sel", [P, E], f32, kind="ExternalInput").ap()
    u128_d = nc.dram_tensor("u128", [P, P], f32, kind="ExternalInput").ap()
    u32_d = nc.dram_tensor("u32", [NT, NT], f32, kind="ExternalInput").ap()
    idf_d = nc.dram_tensor("idf", [P, P], f32, kind="ExternalInput").ap()
    idb_d = nc.dram_tensor("idb", [P, P], bf, kind="ExternalInput").ap()
    iota_d = nc.dram_tensor("iota", [P, NT], f32, kind="ExternalInput").ap()

    lg_out = nc.dram_tensor("logits_out", [T, E], f32, kind="ExternalOutput").ap()
    y_out = nc.dram_tensor("y_out", [CAP, D], f32, kind="ExternalOutput").ap()
    metas = [(nc.dram_tensor(f"meta_q{r}a", [REG, 2], f32).ap(),
              nc.dram_tensor(f"meta_q{r}b", [REG, 2], f32).ap())
             for r in range(4)]

    with tile.TileContext(nc) as tc:
        from contextlib import ExitStack
        with ExitStack() as ctx:
            consts = ctx.enter_context(tc.tile_pool(name="consts", bufs=1))
            wpool = ctx.enter_context(tc.tile_pool(name="wpool", bufs=1))
            xtp = ctx.enter_context(tc.tile_pool(name="xtp", bufs=5))
            rsm = ctx.enter_context(tc.tile_pool(name="rsm", bufs=2))
            asmp = ctx.enter_context(tc.tile_pool(name="asm", bufs=1))
            gthp = ctx.enter_context(tc.tile_pool(name="gth", bufs=4))
            xgp = ctx.enter_context(tc.tile_pool(name="xgp", bufs=2))
            mlpp = ctx.enter_context(tc.tile_pool(name="mlp", bufs=2))
            ytp = ctx.enter_context(tc.tile_pool(name="ytp", bufs=2))
            psum_mm = ctx.enter_context(tc.tile_pool(name="psmm", bufs=4, space="PSUM"))
            psum_sm = ctx.enter_context(tc.tile_pool(name="pssm", bufs=2, space="PSUM"))
            psum_tr = ctx.enter_context(tc.tile_pool(name="pstr", bufs=2, space="PSUM"))

            # ---- router-critical constant ----
            wr_sb = consts.tile([P, KO, E], f32)
            nc.sync.dma_start(wr_sb, wr_d.rearrange("(ko p) e -> p ko e", p=P))
            idf_sb = consts.tile([P, P], f32)
            nc.scalar.dma_start(idf_sb, idf_d)

            # ---- router (fp32) + per-half routing/compaction ----
            lgall = asmp.tile([P, NT, E], f32)    # logits, token t = c*128 + p
            l12 = asmp.tile([P, NT, 8], f32)      # max8 sorted logits per tile
            w_all = asmp.tile([P, NT], f32)
            m_all = asmp.tile([P, NT], f32)
            pos_i = asmp.tile([P, NT], i32)
            sc = asmp.tile([P, NT, 2], f32)
            mz = asmp.tile([64, 2 * REG // 64], f32)
            xt_re = xt_d.rearrange("(ko p) t -> p ko t", p=P)
            TQ = 1024   # tokens per router sweep (row-contiguous DMA)

            def router_quarter(q):
                pss = [psum_mm.tile([E, 512], f32, tag="mm", name=f"pslt{q}_{th}")
                       for th in range(TQ // 512)]
                for ko in range(KO):
                    xt_g = xtp.tile([P, TQ], f32, tag="xtg", name=f"xtg{q}_{ko}")
                    nc.sync.dma_start(xt_g, xt_re[:, ko, q * TQ:(q + 1) * TQ])
                    for th in range(TQ // 512):
                        nc.tensor.matmul(pss[th], lhsT=wr_sb[:, ko],
                                         rhs=xt_g[:, th * 512:(th + 1) * 512],
                                         start=(ko == 0), stop=(ko == KO - 1))
                for th in range(TQ // 512):
                    lgt = rsm.tile([E, 512], f32, tag="lgt")
                    nc.vector.tensor_copy(lgt, pss[th])
                    for jj in range(4):
                        j = (q * TQ + th * 512) // P + jj
                        ps_l = psum_sm.tile([P, E], f32, tag="small")
                        nc.tensor.transpose(ps_l, lgt[:, jj * P:(jj + 1) * P],
                                            idf_sb[:E, :E])
                        nc.vector.tensor_copy(lgall[:, j], ps_l)
                        nc.vector.max(l12[:, j], lgall[:, j])

            NH = NT // 4

            def route_half(h):
                cs = slice(h * NH, (h + 1) * NH)
                l1 = l12[:, cs, 0]
                l2 = l12[:, cs, 1]
                d21 = rsm.tile([P, NH], f32, tag="d21")
                nc.vector.tensor_sub(d21, l2, l1)
                w2a = rsm.tile([P, NH], f32, tag="w2a")
                nc.scalar.activation(w2a, d21, AF.Sigmoid)   # w2 = sig(l2 - l1)
                w1a = rsm.tile([P, NH], f32, tag="w1a")
                nc.vector.tensor_scalar(w1a, w2a, -1.0, 1.0, op0=OP.mult, op1=OP.add)
                msel = rsm.tile([P, NH, E], f32, tag="msel")
                nc.vector.tensor_mul(msel, lgall[:, cs],
                                     sel_sb[:, None, :].to_broadcast([P, NH, E]))
                le = rsm.tile([P, NH], f32, tag="le")
                nc.vector.reduce_sum(le, msel, axis=AX.X)
                m1 = rsm.tile([P, NH], f32, tag="m1")
                nc.vector.tensor_tensor(m1, le, l1, op=OP.is_equal)
                m2 = rsm.tile([P, NH], f32, tag="m2")
                nc.vector.tensor_tensor(m2, le, l2, op=OP.is_equal)
                t1 = rsm.tile([P, NH], f32, tag="t1")
                nc.vector.tensor_mul(t1, m1, w1a)
                t2 = rsm.tile([P, NH], f32, tag="t2")
                nc.vector.tensor_mul(t2, m2, w2a)
                nc.vector.tensor_add(w_all[:, cs], t1, t2)
                nc.vector.tensor_add(m_all[:, cs], m1, m2)
                nc.vector.tensor_scalar_min(m_all[:, cs], m_all[:, cs], 1.0)
                nc.vector.tensor_copy(sc[:, cs, 1], w_all[:, cs])
                # within-column exclusive cumsum (transposed) + column offsets
                ps_se = psum_sm.tile([NH, P], f32, tag="small")
                nc.tensor.matmul(ps_se, lhsT=m_all[:, cs], rhs=u128_sb,
                                 start=True, stop=True)
                se_sb = rsm.tile([NH, P], f32, tag="sesb")
                nc.vector.tensor_copy(se_sb, ps_se)
                ps_mt = psum_sm.tile([NH, P], f32, tag="small")
                nc.tensor.transpose(ps_mt, m_all[:, cs], idf_sb)
                mt = rsm.tile([NH, P], f32, tag="mt")
                nc.vector.tensor_copy(mt, ps_mt)
                csum_h = rsm.tile([NH, 1], f32, tag="csum")
                nc.vector.reduce_sum(csum_h, mt, axis=AX.X)
                ps_off = psum_sm.tile([NH, 1], f32, tag="small")
                nc.tensor.matmul(ps_off, lhsT=u32_sb[:NH, :NH], rhs=csum_h,
                                 start=True, stop=True)
                offs_h = rsm.tile([NH, 1], f32, tag="offs")
                nc.vector.tensor_copy(offs_h, ps_off)
                posT = rsm.tile([NH, P], f32, tag="posT")
                nc.vector.tensor_scalar_add(posT, se_sb, offs_h)
                ps_pos = psum_sm.tile([P, NH], f32, tag="small")
                nc.tensor.transpose(ps_pos, posT, idf_sb[:NH, :NH])
                pm = rsm.tile([P, NH], f32, tag="pm")
                nc.vector.tensor_mul(pm, ps_pos, m_all[:, cs])
                bigt = rsm.tile([P, NH], f32, tag="bigt")
                nc.vector.tensor_scalar(bigt, m_all[:, cs], -float(BIG), float(BIG),
                                        op0=OP.mult, op1=OP.add)
                nc.vector.tensor_add(pm, pm, bigt)
                nc.vector.tensor_copy(pos_i[:, cs], pm)
                # compact-slot scatter of (token_id, weight)
                with nc.allow_non_contiguous_dma(reason="8B-row compaction scatter"):
                    for c in range(h * NH, (h + 1) * NH):
                        nc.gpsimd.indirect_dma_start(
                            out=metas[h][c % 2],
                            out_offset=IndirectOffsetOnAxis(
                                ap=pos_i[:, c:c + 1], axis=0),
                            in_=sc[:, c, :],
                            in_offset=None,
                            bounds_check=REG - 1,
                            oob_is_err=False,
                        )

            router_quarter(0)

            # remaining constants (off the router-warmup critical path)
            sel_sb = consts.tile([P, E], f32)
            nc.sync.dma_start(sel_sb, sel_d)
            u128_sb = consts.tile([P, P], f32)
            nc.sync.dma_start(u128_sb, u128_d)
            u32_sb = consts.tile([NT, NT], f32)
            nc.sync.dma_start(u32_sb, u32_d)
            idb_sb = consts.tile([P, P], bf)
            nc.sync.dma_start(idb_sb, idb_d)
            iota_sb = consts.tile([P, NT], f32)
            nc.vector.memset(mz, 0.0)
            nc.sync.dma_start(iota_sb, iota_d)
            for r in range(4):
                for mq in metas[r]:
                    nc.scalar.dma_start(
                        mq.rearrange("(o p) b -> p o b", p=64),
                        mz[:64].rearrange("p (o b) -> p o b", b=2))
            nc.vector.tensor_copy(sc[:, :, 0], iota_sb)

            router_quarter(1)
            route_half(0)       # each block overlaps the next router sweep
            router_quarter(2)
            route_half(1)
            router_quarter(3)
            route_half(2)

            # ---- expert weight preload (bf16, stays resident) ----
            wg_sb = wpool.tile([P, KO, F], bf)
            nc.scalar.dma_start(wg_sb, wg_d.rearrange("(ko p) f -> p ko f", p=P))
            wi_sb = wpool.tile([P, KO, F], bf)
            nc.scalar.dma_start(wi_sb, wi_d.rearrange("(ko p) f -> p ko f", p=P))
            wo_sb = wpool.tile([P, KI, D], bf)
            nc.scalar.dma_start(wo_sb, wo_d.rearrange("(ki p) d -> p ki d", p=P))

            # logits output (token-major [T, E])
            nc.scalar.dma_start(lg_out.rearrange("(c p) e -> p c e", p=P), lgall)

            route_half(3)

            # ---- expert MLP over capacity tiles ----
            wv = asmp.tile([P, NJ], f32)   # per-slot routing weight
            for g, (t0, ng) in enumerate(GROUPS):
                njg = ng // P
                xtg = xgp.tile([P, KO, 512], bf, tag="xtgrp")
                for jj in range(njg):
                    j = t0 // P + jj
                    meta_t = gthp.tile([P, 2], f32, tag="meta")
                    meta_t2 = gthp.tile([P, 2], f32, tag="meta2")
                    s0, s1 = j * P, (j + 1) * P
                    with nc.allow_non_contiguous_dma(reason="8B meta rows"):
                        for r in range(s0 // REG, (s1 - 1) // REG + 1):
                            l0 = max(s0, r * REG) - r * REG
                            l1 = min(s1, (r + 1) * REG) - r * REG
                            o0 = r * REG + l0 - s0
                            nc.sync.dma_start(meta_t[o0:o0 + l1 - l0, :],
                                              metas[r][0][l0:l1, :])
                            nc.sync.dma_start(meta_t2[o0:o0 + l1 - l0, :],
                                              metas[r][1][l0:l1, :])
                    nc.vector.tensor_add(meta_t, meta_t, meta_t2)
                    idx_i = gthp.tile([P, 1], i32, tag="idx")
                    nc.vector.tensor_copy(idx_i, meta_t[:, 0:1])
                    nc.vector.tensor_copy(wv[:, j:j + 1], meta_t[:, 1:2])
                    xg = gthp.tile([P, D], bf, tag="xg")
                    nc.gpsimd.indirect_dma_start(
                        out=xg, out_offset=None, in_=xb_d,
                        in_offset=IndirectOffsetOnAxis(ap=idx_i[:, 0:1], axis=0))
                    for ko in range(KO):
                        ps_tr = psum_tr.tile([P, P], bf, tag="trb")
                        nc.tensor.transpose(ps_tr, xg[:, ko * P:(ko + 1) * P], idb_sb)
                        nc.vector.tensor_copy(xtg[:, ko, jj * P:(jj + 1) * P], ps_tr)

                # gate/up + silu + mul (act kept in gsil, bf16)
                gsil = mlpp.tile([P, KI, 512], bf, tag="gsil")
                for m in range(KI):
                    ps = psum_mm.tile([P, 512], f32, tag="mm")
                    for ko in range(KO):
                        nc.tensor.matmul(ps[:, :ng], lhsT=wg_sb[:, ko, m * P:(m + 1) * P],
                                         rhs=xtg[:, ko, :ng], start=(ko == 0),
                                         stop=(ko == KO - 1))
                    nc.scalar.activation(gsil[:, m, :ng], ps[:, :ng], AF.Sigmoid)
                    nc.vector.tensor_mul(gsil[:, m, :ng], ps[:, :ng], gsil[:, m, :ng])
                    ps2 = psum_mm.tile([P, 512], f32, tag="mm")
                    for ko in range(KO):
                        nc.tensor.matmul(ps2[:, :ng], lhsT=wi_sb[:, ko, m * P:(m + 1) * P],
                                         rhs=xtg[:, ko, :ng], start=(ko == 0),
                                         stop=(ko == KO - 1))
                    nc.vector.tensor_mul(gsil[:, m, :ng], ps2[:, :ng], gsil[:, m, :ng])

                # down proj + transpose back + per-token scale + store
                for do in range(KO):
                    ps3 = psum_mm.tile([P, 512], f32, tag="mm")
                    for ki in range(KI):
                        nc.tensor.matmul(ps3[:, :ng], lhsT=wo_sb[:, ki, do * P:(do + 1) * P],
                                         rhs=gsil[:, ki, :ng], start=(ki == 0),
                                         stop=(ki == KI - 1))
                    ysb = ytp.tile([P, 512], f32, tag="ysb")
                    nc.vector.tensor_copy(ysb[:, :ng], ps3[:, :ng])
                    for jj in range(njg):
                        j = t0 // P + jj
                        ps4 = psum_sm.tile([P, P], f32, tag="small")
                        nc.tensor.transpose(ps4, ysb[:, jj * P:(jj + 1) * P], idf_sb)
                        ystg = ytp.tile([P, P], f32, tag="ystg")
                        nc.vector.tensor_scalar_mul(ystg, ps4, wv[:, j:j + 1])
                        nc.sync.dma_start(
                            y_out[j * P:(j + 1) * P, do * P:(do + 1) * P], ystg)

    nc.compile()
    return nc


def _get_nc():
    if "nc" not in _CACHE:
        _CACHE["nc"] = _build_nc()
    return _CACHE["nc"]


def _make_in_maps(x, W_router, W_gate, W_in, W_out):
    bf16 = ml_dtypes.bfloat16
    x2d = np.ascontiguousarray(x.reshape(T, D).astype(np.float32))
    x_t = np.ascontiguousarray(x2d.T)
    x_bf = np.ascontiguousarray(x2d.astype(bf16))
    wr = np.ascontiguousarray(W_router.astype(np.float32))
    u128 = np.triu(np.ones((P, P), np.float32), 1)
    u32 = np.triu(np.ones((NT, NT), np.float32), 1)
    idf = np.eye(P, dtype=np.float32)
    idb = np.eye(P, dtype=np.float32).astype(bf16)
    iota = np.ascontiguousarray(
        (np.arange(P, dtype=np.float32)[:, None]
         + P * np.arange(NT, dtype=np.float32)[None, :]).astype(np.float32))

    in_maps = []
    for e in range(N_CORES):
        sel = np.zeros((P, E), np.float32)
        sel[:, e] = 1.0
        in_maps.append({
            "x_t": x_t,
            "x_bf": x_bf,
            "w_r": wr,
            "w_g": np.ascontiguousarray(W_gate[e].astype(bf16)),
            "w_i": np.ascontiguousarray(W_in[e].astype(bf16)),
            "w_o": np.ascontiguousarray(W_out[e].astype(bf16)),
            "sel": sel,
            "u128": u128,
            "u32": u32,
            "idf": idf,
            "idb": idb,
            "iota": iota,
        })
    return in_maps


def kernel(x, W_router, W_gate, W_in, W_out, _trace=False, _trace_cores=None):
    from concourse.bass_utils import run_bass_kernel_spmd

    nc = _get_nc()
    in_maps = _make_in_maps(x, W_router, W_gate, W_in, W_out)
    res = run_bass_kernel_spmd(nc, in_maps, list(range(N_CORES)),
                               trace=_trace, trace_cores=_trace_cores)
    kernel._last_results = res

    logits = np.asarray(res.results[0]["logits_out"], dtype=np.float32)
    # Replicate the device's top-2 membership mask bit-exactly from the same
    # fp32 logits the device routed with; slot order == ascending token id.
    srt = np.sort(logits, axis=1)[:, ::-1]
    l1, l2 = srt[:, 0:1], srt[:, 1:2]
    member = (logits == l1) | (logits == l2)      # [T, E]
    out = np.zeros((T, D), np.float32)
    for e in range(N_CORES):
        y = np.asarray(res.results[e]["y_out"])
        for r in range(4):
            mem_r = member[r * (T // 4):(r + 1) * (T // 4), e]
            idx = np.nonzero(mem_r)[0][:REG] + r * (T // 4)
            out[idx] += y[r * REG:r * REG + len(idx)]
    B, S = 2, 2048
    return out.reshape(B, S, D), logits


# revision 42
# speedup vs baseline: 1.0513x; 1.0394x over previous
"""MoE (top-2 of 8 experts, D=1024, F=2048, T=4096) on 8 Trainium2 NeuronCores.

Strategy: expert-parallel. Every core replicates the fp32 router over all
4096 tokens, selects the tokens routed to ITS expert (top-2 membership via
max8 on logits; weights w1=sigmoid(l1-l2) renormalized pair weights),
compacts their indices with a matmul-based exclusive cumsum + indirect-DMA
scatter, gathers those token rows, runs the gated-MLP for its single expert
in bf16 (fp32 accumulate), scales each token's output row by its routing
weight, and writes a compact [CAP, D] fp32 result + the slot->token map.
The host sums the 8 compact shards into the full [T, D] output. Router
logits are computed in fp32 on-device and returned from core 0.
"""

import os
import sys

import numpy as np
import ml_dtypes

if "/opt/trn_rl_repo" not in sys.path:
    sys.path.insert(0, "/opt/trn_rl_repo")

# Problem shapes (hardcoded per contract)
T, D, F, E = 4096, 1024, 2048, 8
P = 128
NT = T // P            # 32 token tiles of 128
REG = 320              # slots per token-quarter region (mean 256, ~4.5 sigma margin)
CAP = 4 * REG          # per-expert token capacity
NJ = CAP // P          # capacity tiles
GROUPS = [(0, 512), (512, 512), (1024, 256)]  # (slot offset, group size)
KO = D // P            # 8 contraction chunks over D
KI = F // P            # 16 contraction chunks over F
DP = D + 32            # compact row width, 64B-aligned (w_hi/w_lo at D, D+1)
BIG = 100000           # position sentinel for unselected tokens
                       # (BIG * row-stride must stay well inside int32)

N_CORES = 8

_CACHE = {}


def _build_nc():
    import concourse.tile as tile
    from concourse import bacc, mybir
    from concourse.bass import IndirectOffsetOnAxis

    f32 = mybir.dt.float32
    bf = mybir.dt.bfloat16
    i32 = mybir.dt.int32
    AF = mybir.ActivationFunctionType
    AX = mybir.AxisListType
    OP = mybir.AluOpType

    nc = bacc.Bacc("TRN2", target_bir_lowering=False, debug=False,
                   enable_asserts=False, num_devices=N_CORES)

    # ---- I/O ----
    xt_d = nc.dram_tensor("x_t", [D, T], f32, kind="ExternalInput").ap()
    xb_d = nc.dram_tensor("x_bf", [T, D], bf, kind="ExternalInput").ap()
    wr_d = nc.dram_tensor("w_r", [D, E], f32, kind="ExternalInput").ap()
    wg_d = nc.dram_tensor("w_g", [D, F], bf, kind="ExternalInput").ap()
    wi_d = nc.dram_tensor("w_i", [D, F], bf, kind="ExternalInput").ap()
    wo_d = nc.dram_tensor("w_o", [F, D], bf, kind="ExternalInput").ap()
    sel_d = nc.dram_tensor("sel", [P, E], f32, kind="ExternalInput").ap()
    u128_d = nc.dram_tensor("u128", [P, P], f32, kind="ExternalInput").ap()
    u32_d = nc.dram_tensor("u32", [NT, NT], f32, kind="ExternalInput").ap()
    idf_d = nc.dram_tensor("idf", [P, P], f32, kind="ExternalInput").ap()
    idb_d = nc.dram_tensor("idb", [P, P], bf, kind="ExternalInput").ap()
    iota_d = nc.dram_tensor("iota", [P, NT], f32, kind="ExternalInput").ap()

    lg_out = nc.dram_tensor("logits_out", [T, E], f32, kind="ExternalOutput").ap()
    y_out = nc.dram_tensor("y_out", [CAP, D], f32, kind="ExternalOutput").ap()
    metas = [(nc.dram_tensor(f"meta_q{r}a", [REG, 2], f32).ap(),
              nc.dram_tensor(f"meta_q{r}b", [REG, 2], f32).ap())
             for r in range(4)]

    with tile.TileContext(nc) as tc:
        from contextlib import ExitStack
        with ExitStack() as ctx:
            consts = ctx.enter_context(tc.tile_pool(name="consts", bufs=1))
            wpool = ctx.enter_context(tc.tile_pool(name="wpool", bufs=1))
            xtp = ctx.enter_context(tc.tile_pool(name="xtp", bufs=5))
            rsm = ctx.enter_context(tc.tile_pool(name="rsm", bufs=2))
            asmp = ctx.enter_context(tc.tile_pool(name="asm", bufs=1))
            gthp = ctx.enter_context(tc.tile_pool(name="gth", bufs=4))
            xgp = ctx.enter_context(tc.tile_pool(name="xgp", bufs=2))
            mlpp = ctx.enter_context(tc.tile_pool(name="mlp", bufs=2))
            ytp = ctx.enter_context(tc.tile_pool(name="ytp", bufs=2))
            psum_mm = ctx.enter_context(tc.tile_pool(name="psmm", bufs=4, space="PSUM"))
            psum_sm = ctx.enter_context(tc.tile_pool(name="pssm", bufs=2, space="PSUM"))
            psum_tr = ctx.enter_context(tc.tile_pool(name="pstr", bufs=2, space="PSUM"))

            # ---- router-critical constant ----
            wr_sb = consts.tile([P, KO, E], f32)
            nc.sync.dma_start(wr_sb, wr_d.rearrange("(ko p) e -> p ko e", p=P))
            idf_sb = consts.tile([P, P], f32)
            nc.scalar.dma_start(idf_sb, idf_d)

            # ---- router (fp32) + per-half routing/compaction ----
            lgall = asmp.tile([P, NT, E], f32)    # logits, token t = c*128 + p
            l12 = asmp.tile([P, NT, 8], f32)      # max8 sorted logits per tile
            w_all = asmp.tile([P, NT], f32)
            m_all = asmp.tile([P, NT], f32)
            pos_i = asmp.tile([P, NT], i32)
            sc = asmp.tile([P, NT, 2], f32)
            mz = asmp.tile([64, 2 * REG // 64], f32)
            xt_re = xt_d.rearrange("(ko p) t -> p ko t", p=P)
            TQ = 1024   # tokens per router sweep (row-contiguous DMA)

            def router_quarter(q):
                pss = [psum_mm.tile([E, 512], f32, tag="mm", name=f"pslt{q}_{th}")
                       for th in range(TQ // 512)]
                for ko in range(KO):
                    xt_g = xtp.tile([P, TQ], f32, tag="xtg", name=f"xtg{q}_{ko}")
                    nc.sync.dma_start(xt_g, xt_re[:, ko, q * TQ:(q + 1) * TQ])
                    for th in range(TQ // 512):
                        nc.tensor.matmul(pss[th], lhsT=wr_sb[:, ko],
                                         rhs=xt_g[:, th * 512:(th + 1) * 512],
                                         start=(ko == 0), stop=(ko == KO - 1))
                for th in range(TQ // 512):
                    lgt = rsm.tile([E, 512], f32, tag="lgt")
                    nc.vector.tensor_copy(lgt, pss[th])
                    for jj in range(4):
                        j = (q * TQ + th * 512) // P + jj
                        ps_l = psum_sm.tile([P, E], f32, tag="small")
                        nc.tensor.transpose(ps_l, lgt[:, jj * P:(jj + 1) * P],
                                            idf_sb[:E, :E])
                        nc.vector.tensor_copy(lgall[:, j], ps_l)
                        nc.vector.max(l12[:, j], lgall[:, j])

            NH = NT // 4

            def route_half(h):
                cs = slice(h * NH, (h + 1) * NH)
                l1 = l12[:, cs, 0]
                l2 = l12[:, cs, 1]
                d21 = rsm.tile([P, NH], f32, tag="d21")
                nc.vector.tensor_sub(d21, l2, l1)
                w2a = rsm.tile([P, NH], f32, tag="w2a")
                nc.scalar.activation(w2a, d21, AF.Sigmoid)   # w2 = sig(l2 - l1)
                w1a = rsm.tile([P, NH], f32, tag="w1a")
                nc.vector.tensor_scalar(w1a, w2a, -1.0, 1.0, op0=OP.mult, op1=OP.add)
                msel = rsm.tile([P, NH, E], f32, tag="msel")
                nc.vector.tensor_mul(msel, lgall[:, cs],
                                     sel_sb[:, None, :].to_broadcast([P, NH, E]))
                le = rsm.tile([P, NH], f32, tag="le")
                nc.vector.reduce_sum(le, msel, axis=AX.X)
                m1 = rsm.tile([P, NH], f32, tag="m1")
                nc.vector.tensor_tensor(m1, le, l1, op=OP.is_equal)
                m2 = rsm.tile([P, NH], f32, tag="m2")
                nc.vector.tensor_tensor(m2, le, l2, op=OP.is_equal)
                t1 = rsm.tile([P, NH], f32, tag="t1")
                nc.vector.tensor_mul(t1, m1, w1a)
                t2 = rsm.tile([P, NH], f32, tag="t2")
                nc.vector.tensor_mul(t2, m2, w2a)
                nc.vector.tensor_add(w_all[:, cs], t1, t2)
                nc.vector.tensor_add(m_all[:, cs], m1, m2)
                nc.vector.tensor_scalar_min(m_all[:, cs], m_all[:, cs], 1.0)
                nc.vector.tensor_copy(sc[:, cs, 1], w_all[:, cs])
                # within-column exclusive cumsum (transposed) + column offsets
                ps_se = psum_sm.tile([NH, P], f32, tag="small")
                nc.tensor.matmul(ps_se, lhsT=m_all[:, cs], rhs=u128_sb,
                                 start=True, stop=True)
                se_sb = rsm.tile([NH, P], f32, tag="sesb")
                nc.vector.tensor_copy(se_sb, ps_se)
                ps_mt = psum_sm.tile([NH, P], f32, tag="small")
                nc.tensor.transpose(ps_mt, m_all[:, cs], idf_sb)
                mt = rsm.tile([NH, P], f32, tag="mt")
                nc.vector.tensor_copy(mt, ps_mt)
                csum_h = rsm.tile([NH, 1], f32, tag="csum")
                nc.vector.reduce_sum(csum_h, mt, axis=AX.X)
                ps_off = psum_sm.tile([NH, 1], f32, tag="small")
                nc.tensor.matmul(ps_off, lhsT=u32_sb[:NH, :NH], rhs=csum_h,
                                 start=True, stop=True)
                offs_h = rsm.tile([NH, 1], f32, tag="offs")
                nc.vector.tensor_copy(offs_h, ps_off)
                posT = rsm.tile([NH, P], f32, tag="posT")
                nc.vector.tensor_scalar_add(posT, se_sb, offs_h)
                ps_pos = psum_sm.tile([P, NH], f32, tag="small")
                nc.tensor.transpose(ps_pos, posT, idf_sb[:NH, :NH])
                pm = rsm.tile([P, NH], f32, tag="pm")
                nc.vector.tensor_mul(pm, ps_pos, m_all[:, cs])
                bigt = rsm.tile([P, NH], f32, tag="bigt")
                nc.vector.tensor_scalar(bigt, m_all[:, cs], -float(BIG), float(BIG),
                                        op0=OP.mult, op1=OP.add)
                nc.vector.tensor_add(pm, pm, bigt)
                nc.vector.tensor_copy(pos_i[:, cs], pm)
                # compact-slot scatter of (token_id, weight)
                with nc.allow_non_contiguous_dma(reason="8B-row compaction scatter"):
                    for c in range(h * NH, (h + 1) * NH):
                        nc.gpsimd.indirect_dma_start(
                            out=metas[h][c % 2],
                            out_offset=IndirectOffsetOnAxis(
                                ap=pos_i[:, c:c + 1], axis=0),
                            in_=sc[:, c, :],
                            in_offset=None,
                            bounds_check=REG - 1,
                            oob_is_err=False,
                        )

            router_quarter(0)

            # remaining constants (off the router-warmup critical path)
            sel_sb = consts.tile([P, E], f32)
            nc.sync.dma_start(sel_sb, sel_d)
            u128_sb = consts.tile([P, P], f32)
            nc.sync.dma_start(u128_sb, u128_d)
            u32_sb = consts.tile([NT, NT], f32)
            nc.sync.dma_start(u32_sb, u32_d)
            idb_sb = consts.tile([P, P], bf)
            nc.sync.dma_start(idb_sb, idb_d)
            iota_sb = consts.tile([P, NT], f32)
            nc.vector.memset(mz, 0.0)
            nc.sync.dma_start(iota_sb, iota_d)
            for r in range(4):
                for mq in metas[r]:
                    nc.scalar.dma_start(
                        mq.rearrange("(o p) b -> p o b", p=64),
                        mz[:64].rearrange("p (o b) -> p o b", b=2))
            nc.vector.tensor_copy(sc[:, :, 0], iota_sb)

            route_half(0)       # each block right after its own quarter
            router_quarter(1)
            route_half(1)
            router_quarter(2)
            route_half(2)
            router_quarter(3)
            route_half(3)

            # ---- expert weight preload (bf16, stays resident) ----
            wg_sb = wpool.tile([P, KO, F], bf)
            nc.scalar.dma_start(wg_sb, wg_d.rearrange("(ko p) f -> p ko f", p=P))
            wi_sb = wpool.tile([P, KO, F], bf)
            nc.scalar.dma_start(wi_sb, wi_d.rearrange("(ko p) f -> p ko f", p=P))
            wo_sb = wpool.tile([P, KI, D], bf)
            nc.scalar.dma_start(wo_sb, wo_d.rearrange("(ki p) d -> p ki d", p=P))

            # logits output (token-major [T, E])
            nc.scalar.dma_start(lg_out.rearrange("(c p) e -> p c e", p=P), lgall)

            # ---- expert MLP over capacity tiles ----
            wv = asmp.tile([P, NJ], f32)   # per-slot routing weight
            for g, (t0, ng) in enumerate(GROUPS):
                njg = ng // P
                xtg = xgp.tile([P, KO, 512], bf, tag="xtgrp")
                for jj in range(njg):
                    j = t0 // P + jj
                    meta_t = gthp.tile([P, 2], f32, tag="meta")
                    meta_t2 = gthp.tile([P, 2], f32, tag="meta2")
                    s0, s1 = j * P, (j + 1) * P
                    with nc.allow_non_contiguous_dma(reason="8B meta rows"):
                        for r in range(s0 // REG, (s1 - 1) // REG + 1):
                            l0 = max(s0, r * REG) - r * REG
                            l1 = min(s1, (r + 1) * REG) - r * REG
                            o0 = r * REG + l0 - s0
                            nc.sync.dma_start(meta_t[o0:o0 + l1 - l0, :],
                                              metas[r][0][l0:l1, :])
                            nc.sync.dma_start(meta_t2[o0:o0 + l1 - l0, :],
                                              metas[r][1][l0:l1, :])
                    nc.vector.tensor_add(meta_t, meta_t, meta_t2)
                    idx_i = gthp.tile([P, 1], i32, tag="idx")
                    nc.vector.tensor_copy(idx_i, meta_t[:, 0:1])
                    nc.vector.tensor_copy(wv[:, j:j + 1], meta_t[:, 1:2])
                    xg = gthp.tile([P, D], bf, tag="xg")
                    nc.gpsimd.indirect_dma_start(
                        out=xg, out_offset=None, in_=xb_d,
                        in_offset=IndirectOffsetOnAxis(ap=idx_i[:, 0:1], axis=0))
                    for ko in range(KO):
                        ps_tr = psum_tr.tile([P, P], bf, tag="trb")
                        nc.tensor.transpose(ps_tr, xg[:, ko * P:(ko + 1) * P], idb_sb)
                        nc.vector.tensor_copy(xtg[:, ko, jj * P:(jj + 1) * P], ps_tr)

                # gate/up + silu + mul (act kept in gsil, bf16)
                gsil = mlpp.tile([P, KI, 512], bf, tag="gsil")
                for m in range(KI):
                    ps = psum_mm.tile([P, 512], f32, tag="mm")
                    for ko in range(KO):
                        nc.tensor.matmul(ps[:, :ng], lhsT=wg_sb[:, ko, m * P:(m + 1) * P],
                                         rhs=xtg[:, ko, :ng], start=(ko == 0),
                                         stop=(ko == KO - 1))
                    nc.scalar.activation(gsil[:, m, :ng], ps[:, :ng], AF.Sigmoid)
                    nc.vector.tensor_mul(gsil[:, m, :ng], ps[:, :ng], gsil[:, m, :ng])
                    ps2 = psum_mm.tile([P, 512], f32, tag="mm")
                    for ko in range(KO):
                        nc.tensor.matmul(ps2[:, :ng], lhsT=wi_sb[:, ko, m * P:(m + 1) * P],
                                         rhs=xtg[:, ko, :ng], start=(ko == 0),
                                         stop=(ko == KO - 1))
                    nc.vector.tensor_mul(gsil[:, m, :ng], ps2[:, :ng], gsil[:, m, :ng])

                # down proj + transpose back + per-token scale + store
                for do in range(KO):
                    ps3 = psum_mm.tile([P, 512], f32, tag="mm")
                    for ki in range(KI):
                        nc.tensor.matmul(ps3[:, :ng], lhsT=wo_sb[:, ki, do * P:(do + 1) * P],
                                         rhs=gsil[:, ki, :ng], start=(ki == 0),
                                         stop=(ki == KI - 1))
                    ysb = ytp.tile([P, 512], f32, tag="ysb")
                    nc.vector.tensor_copy(ysb[:, :ng], ps3[:, :ng])
                    for jj in range(njg):
                        j = t0 // P + jj
                        ps4 = psum_sm.tile([P, P], f32, tag="small")
                        nc.tensor.transpose(ps4, ysb[:, jj * P:(jj + 1) * P], idf_sb)
                        ystg = ytp.tile([P, P], f32, tag="ystg")
                        nc.vector.tensor_scalar_mul(ystg, ps4, wv[:, j:j + 1])
                        nc.sync.dma_start(
                            y_out[j * P:(j + 1) * P, do * P:(do + 1) * P], ystg)

    nc.compile()
    return nc


def _get_nc():
    if "nc" not in _CACHE:
        _CACHE["nc"] = _build_nc()
    return _CACHE["nc"]


def _make_in_maps(x, W_router, W_gate, W_in, W_out):
    bf16 = ml_dtypes.bfloat16
    x2d = np.ascontiguousarray(x.reshape(T, D).astype(np.float32))
    x_t = np.ascontiguousarray(x2d.T)
    x_bf = np.ascontiguousarray(x2d.astype(bf16))
    wr = np.ascontiguousarray(W_router.astype(np.float32))
    u128 = np.triu(np.ones((P, P), np.float32), 1)
    u32 = np.triu(np.ones((NT, NT), np.float32), 1)
    idf = np.eye(P, dtype=np.float32)
    idb = np.eye(P, dtype=np.float32).astype(bf16)
    iota = np.ascontiguousarray(
        (np.arange(P, dtype=np.float32)[:, None]
         + P * np.arange(NT, dtype=np.float32)[None, :]).astype(np.float32))

    in_maps = []
    for e in range(N_CORES):
        sel = np.zeros((P, E), np.float32)
        sel[:, e] = 1.0
        in_maps.append({
            "x_t": x_t,
            "x_bf": x_bf,
            "w_r": wr,
            "w_g": np.ascontiguousarray(W_gate[e].astype(bf16)),
            "w_i": np.ascontiguousarray(W_in[e].astype(bf16)),
            "w_o": np.ascontiguousarray(W_out[e].astype(bf16)),
            "sel": sel,
            "u128": u128,
            "u32": u32,
            "idf": idf,
            "idb": idb,
            "iota": iota,
        })
    return in_maps


def kernel(x, W_router, W_gate, W_in, W_out, _trace=False, _trace_cores=None):
    from concourse.bass_utils import run_bass_kernel_spmd

    nc = _get_nc()
    in_maps = _make_in_maps(x, W_router, W_gate, W_in, W_out)
    res = run_bass_kernel_spmd(nc, in_maps, list(range(N_CORES)),
                               trace=_trace, trace_cores=_trace_cores)
    kernel._last_results = res

    logits = np.asarray(res.results[0]["logits_out"], dtype=np.float32)
    # Replicate the device's top-2 membership mask bit-exactly from the same
    # fp32 logits the device routed with; slot order == ascending token id.
    srt = np.sort(logits, axis=1)[:, ::-1]
    l1, l2 = srt[:, 0:1], srt[:, 1:2]
    member = (logits == l1) | (logits == l2)      # [T, E]
    out = np.zeros((T, D), np.float32)
    for e in range(N_CORES):
        y = np.asarray(res.results[e]["y_out"])
        for r in range(4):
            mem_r = member[r * (T // 4):(r + 1) * (T // 4), e]
            idx = np.nonzero(mem_r)[0][:REG] + r * (T // 4)
            out[idx] += y[r * REG:r * REG + len(idx)]
    B, S = 2, 2048
    return out.reshape(B, S, D), logits
